# revision 7
# baseline (speedup 1.0000x reference)
"""Deformable-conv (depth-aware) Trainium2 kernel.

Sharding: pure data parallel — 8 cores = 2 images x 4 H-strips of 32 rows.
Each core computes its strip's output from per-image gather-record tables.

Device algorithm per core (strip of 32 rows x 128 cols = 4096 pixels, 9
samples each):
  1. offset conv (PE): off[pix, 18] = sum_k x_slice @ w_p_k   (K=65 incl bias)
  2. pass-1 depth bilinear sampling via dma_gather of 2x2-block records
     (f32), with clamp-corrected row/col weights; depth weights dw, m (ACT exp)
  3. off2 = off * dw; pass-2 coords/weights; final per-corner weights w4 = m*row*col
  4. dma_gather of 2x2x64ch x-records (fp16, channel-major/corner-minor),
     one DVE mul (weights broadcast over channels) + corner-reduce
  5. DMA-transpose to [(n,c), pix] tiles, PE matmul vs w_conv -> out strip
"""
import numpy as np

B, C, H, W = 2, 64, 128, 128
N = 9
WP = W + 2           # 130 padded width
SP = H // 4          # 32 strip rows
NPIX = SP * W        # 4096 pixels per strip
NS = NPIX * N        # 36864 samples per strip
NREC = WP * WP       # 16900 records

_CACHE = {}


# ---------------------------------------------------------------------------
# device program
# ---------------------------------------------------------------------------
def _build_program():
    import concourse.bacc as bacc
    import concourse.tile as tile
    import concourse.mybir as mybir

    dt = mybir.dt
    Alu = mybir.AluOpType
    Act = mybir.ActivationFunctionType

    nc = bacc.Bacc("TRN2", target_bir_lowering=False, debug=False,
                   enable_asserts=False, num_devices=8)

    xs_d = nc.dram_tensor("xs", [65, 34 * WP], dt.float32, kind="ExternalInput")
    r2_d = nc.dram_tensor("r2", [NREC, 256], dt.float16, kind="ExternalInput")
    r1_d = nc.dram_tensor("r1", [NREC, 64], dt.float32, kind="ExternalInput")
    base_d = nc.dram_tensor("base", [128, 32 * 18], dt.float32, kind="ExternalInput")
    dcen_d = nc.dram_tensor("dcen", [128, 32], dt.float32, kind="ExternalInput")
    wp_d = nc.dram_tensor("wp", [65, 9 * 18], dt.float32, kind="ExternalInput")
    w2_d = nc.dram_tensor("w2", [128, 5 * 64], dt.float16, kind="ExternalInput")
    out_d = nc.dram_tensor("o", [64, NPIX], dt.float32, kind="ExternalOutput")

    with tile.TileContext(nc) as tc:
        with (
            tc.tile_pool(name="const", bufs=1) as cp,
            tc.tile_pool(name="work", bufs=1) as wk,
            tc.tile_pool(name="g1p", bufs=2) as g1p,
            tc.tile_pool(name="g2p", bufs=2) as g2p,
            tc.tile_pool(name="u4p", bufs=1) as u4p,
            tc.tile_pool(name="urp", bufs=2) as urp,
            tc.tile_pool(name="xtp", bufs=2) as xtp,
            tc.tile_pool(name="osp", bufs=2) as osp,
            tc.tile_pool(name="psc", bufs=2, space="PSUM") as psc,
            tc.tile_pool(name="psm", bufs=2, space="PSUM") as psm,
        ):
            f32 = dt.float32
            # ---- load constants
            xs = cp.tile([65, 34, WP], f32, tag="xs")
            nc.sync.dma_start(xs[:], xs_d[:].rearrange("c (a b) -> c a b", b=WP))
            base = cp.tile([128, 32, 18], f32, tag="base")
            nc.sync.dma_start(base[:], base_d[:].rearrange("p (a b) -> p a b", b=18))
            dcen = cp.tile([128, 32], f32, tag="dcen")
            nc.sync.dma_start(dcen[:], dcen_d[:])
            wp = cp.tile([65, 9 * 18], f32, tag="wp")
            nc.sync.dma_start(wp[:], wp_d[:])
            w2 = cp.tile([128, 5 * 64], dt.float16, tag="w2")
            nc.sync.dma_start(w2[:], w2_d[:])

            # ---- stage A: offset conv -> OFF [128, 32, 18]
            OFF = wk.tile([128, 32, 18], f32, tag="OFF")
            for bg in range(8):
                ps = psc.tile([128, 72], f32)
                for bb in range(4):
                    b = bg * 4 + bb
                    for k in range(9):
                        drr, dcc = k // 3, k % 3
                        nc.tensor.matmul(
                            ps[:, bb * 18:(bb + 1) * 18],
                            lhsT=xs[:, b + drr, dcc:dcc + 128],
                            rhs=wp[:, k * 18:(k + 1) * 18],
                            start=(k == 0), stop=(k == 8),
                        )
                nc.scalar.copy(OFF[:, bg * 4:(bg + 1) * 4, :],
                               ps[:].rearrange("p (a b) -> p a b", b=18))

            def sample_math(Pc, bound):
                """Pc: [128, 32, 18] coords. Returns (r0, wA, wB) each
                [128, 32, 18] f32 (x-half rows 0:9, y-half 9:18)."""
                fi = wk.tile([128, 32, 18], dt.int32, tag="sm_fi")
                nc.vector.tensor_copy(fi[:], Pc[:])
                f = wk.tile([128, 32, 18], f32, tag="sm_f")
                nc.vector.tensor_copy(f[:], fi[:])
                gt = wk.tile([128, 32, 18], f32, tag="sm_gt")
                nc.vector.tensor_tensor(gt[:], f[:], Pc[:], Alu.is_gt)
                nc.vector.tensor_sub(f[:], f[:], gt[:])
                qlt = wk.tile([128, 32, 18], f32, tag="sm_qlt")
                nc.vector.tensor_scalar(qlt[:], f[:], 0.0, float(bound - 1), Alu.max, Alu.min)
                qrb = wk.tile([128, 32, 18], f32, tag="sm_qrb")
                nc.vector.tensor_scalar(qrb[:], f[:], 1.0, float(bound - 1), Alu.add, Alu.min)
                nc.vector.tensor_scalar(qrb[:], qrb[:], 0.0, None, Alu.max)
                pc = wk.tile([128, 32, 18], f32, tag="sm_pc")
                nc.vector.tensor_scalar(pc[:], Pc[:], 0.0, float(bound - 1), Alu.max, Alu.min)
                gl = wk.tile([128, 32, 18], f32, tag="sm_gl")
                nc.vector.scalar_tensor_tensor(gl[:], qlt[:], 1.0, pc[:], Alu.add, Alu.subtract)
                gr = wk.tile([128, 32, 18], f32, tag="sm_gr")
                nc.vector.scalar_tensor_tensor(gr[:], pc[:], 1.0, qrb[:], Alu.add, Alu.subtract)
                r0 = wk.tile([128, 32, 18], f32, tag="sm_r0")
                nc.vector.tensor_scalar(r0[:], qlt[:], 0.0, float(bound - 2), Alu.max, Alu.min)
                r0p = wk.tile([128, 32, 18], f32, tag="sm_r0p")
                nc.vector.tensor_scalar(r0p[:], r0[:], 1.0, None, Alu.add)
                eq = wk.tile([128, 32, 18], f32, tag="sm_eq")
                wA = wk.tile([128, 32, 18], f32, tag="sm_wA")
                wB = wk.tile([128, 32, 18], f32, tag="sm_wB")
                tmp = wk.tile([128, 32, 18], f32, tag="sm_tmp")
                # wA = gl*(qlt==r0) + gr*(qrb==r0)
                nc.vector.tensor_tensor(eq[:], qlt[:], r0[:], Alu.is_equal)
                nc.vector.tensor_mul(wA[:], gl[:], eq[:])
                nc.vector.tensor_tensor(eq[:], qrb[:], r0[:], Alu.is_equal)
                nc.vector.tensor_mul(tmp[:], gr[:], eq[:])
                nc.vector.tensor_add(wA[:], wA[:], tmp[:])
                # wB = gl*(qlt==r0+1) + gr*(qrb==r0+1)
                nc.vector.tensor_tensor(eq[:], qlt[:], r0p[:], Alu.is_equal)
                nc.vector.tensor_mul(wB[:], gl[:], eq[:])
                nc.vector.tensor_tensor(eq[:], qrb[:], r0p[:], Alu.is_equal)
                nc.vector.tensor_mul(tmp[:], gr[:], eq[:])
                nc.vector.tensor_add(wB[:], wB[:], tmp[:])
                return r0, wA, wB

            def make_idx(r0, name):
                """idx = r0x*130 + r0y -> int16 [128, 288] + wrapped [128, 288, 8]."""
                idxf = wk.tile([128, 32, 9], f32, tag=name + "_f")
                nc.vector.scalar_tensor_tensor(
                    idxf[:], r0[:, :, 0:9], float(WP), r0[:, :, 9:18],
                    Alu.mult, Alu.add)
                idxi = wk.tile([128, 288], dt.int16, tag=name + "_i")
                nc.vector.tensor_copy(idxi[:], idxf[:].rearrange("p a b -> p (a b)"))
                idxw = wk.tile([128, 288, 8], dt.int16, tag=name + "_w")
                for s in range(8):
                    nc.sync.dma_start(idxw[0:16, :, s], idxi[16 * s:16 * (s + 1), :])
                for r in range(1, 8):
                    nc.sync.dma_start(idxw[16 * r:16 * (r + 1), :, :], idxw[0:16, :, :])
                return idxw

            # ---- stage B: pass-1 coords and weights
            P1 = wk.tile([128, 32, 18], f32, tag="P1")
            nc.vector.tensor_add(P1[:], OFF[:], base[:])
            r0_1, wA1, wB1 = sample_math(P1, H)       # bounds 128
            wT1 = wk.tile([128, 32, 9], f32, tag="wT1")
            nc.vector.tensor_copy(wT1[:], wA1[:, :, 0:9])
            wB1x = wk.tile([128, 32, 9], f32, tag="wB1x")
            nc.vector.tensor_copy(wB1x[:], wB1[:, :, 0:9])
            cL1 = wk.tile([128, 32, 9], f32, tag="cL1")
            nc.vector.tensor_copy(cL1[:], wA1[:, :, 9:18])
            cR1 = wk.tile([128, 32, 9], f32, tag="cR1")
            nc.vector.tensor_copy(cR1[:], wB1[:, :, 9:18])
            idx1w = make_idx(r0_1, "idx1")

            # ---- stage D: pass-1 gathers + blend -> DOFF [128, 32, 9]
            DOFF = wk.tile([128, 32, 9], f32, tag="DOFF")
            for ch in range(8):
                g1 = g1p.tile([128, 36, 64], f32)
                nc.gpsimd.dma_gather(
                    out_ap=g1[:], in_ap=r1_d[:],
                    idxs_ap=idx1w[:, 36 * ch:36 * (ch + 1), :],
                    num_idxs=4608, num_idxs_reg=4608, elem_size=64,
                    single_packet=False)
                sl = slice(4 * ch, 4 * (ch + 1))
                a = wk.tile([128, 4, 9], f32, tag="p1_a")
                bt = wk.tile([128, 4, 9], f32, tag="p1_b")
                t2 = wk.tile([128, 4, 9], f32, tag="p1_t")
                ga = g1[:].rearrange("p (a b) c -> p a b c", b=9)
                nc.vector.tensor_mul(a[:], ga[:, :, :, 0], cL1[:, sl, :])
                nc.vector.tensor_mul(t2[:], ga[:, :, :, 1], cR1[:, sl, :])
                nc.vector.tensor_add(a[:], a[:], t2[:])
                nc.vector.tensor_mul(bt[:], ga[:, :, :, 2], cL1[:, sl, :])
                nc.vector.tensor_mul(t2[:], ga[:, :, :, 3], cR1[:, sl, :])
                nc.vector.tensor_add(bt[:], bt[:], t2[:])
                nc.vector.tensor_mul(a[:], a[:], wT1[:, sl, :])
                nc.vector.tensor_mul(bt[:], bt[:], wB1x[:, sl, :])
                nc.vector.tensor_add(DOFF[:, sl, :], a[:], bt[:])

            # ---- stage E: depth weights
            dd = wk.tile([128, 32, 9], f32, tag="dd")
            nc.vector.tensor_sub(dd[:], dcen[:, :, None].to_broadcast((128, 32, 9)), DOFF[:])
            nc.scalar.activation(dd[:], dd[:], Act.Abs)
            dw = wk.tile([128, 32, 9], f32, tag="dw")
            nc.scalar.activation(dw[:], dd[:], Act.Exp, scale=-4.0)
            nc.vector.tensor_scalar(dw[:], dw[:], 0.25, None, Alu.add)
            mm = wk.tile([128, 32, 9], f32, tag="mm")
            nc.scalar.activation(mm[:], dd[:], Act.Exp, scale=-1.0)

            # ---- stage F: pass-2 coords, weights, indices
            P2 = wk.tile([128, 32, 18], f32, tag="P2")
            nc.vector.tensor_mul(P2[:, :, 0:9], OFF[:, :, 0:9], dw[:])
            nc.vector.tensor_mul(P2[:, :, 9:18], OFF[:, :, 9:18], dw[:])
            nc.vector.tensor_add(P2[:], P2[:], base[:])
            r0_2, wA2, wB2 = sample_math(P2, H + 2)   # bounds 130
            idx2w = make_idx(r0_2, "idx2")
            wTm = wk.tile([128, 32, 9], f32, tag="wTm")
            nc.vector.tensor_mul(wTm[:], wA2[:, :, 0:9], mm[:])
            wBm = wk.tile([128, 32, 9], f32, tag="wBm")
            nc.vector.tensor_mul(wBm[:], wB2[:, :, 0:9], mm[:])
            w4 = wk.tile([128, 288, 4], f32, tag="w4")
            w4v = w4[:].rearrange("p (a b) c -> p a b c", b=9)
            nc.vector.tensor_mul(w4v[:, :, :, 0], wTm[:], wA2[:, :, 9:18])
            nc.vector.tensor_mul(w4v[:, :, :, 1], wTm[:], wB2[:, :, 9:18])
            nc.vector.tensor_mul(w4v[:, :, :, 2], wBm[:], wA2[:, :, 9:18])
            nc.vector.tensor_mul(w4v[:, :, :, 3], wBm[:], wB2[:, :, 9:18])
            w4h = wk.tile([128, 288, 4], dt.float16, tag="w4h")
            nc.vector.tensor_copy(w4h[:], w4[:])

            # ---- stage G: pass-2 gather, blend, transpose, matmul
            for cg in range(8):
                g2 = g2p.tile([128, 36, 256], dt.float16)
                nc.gpsimd.dma_gather(
                    out_ap=g2[:], in_ap=r2_d[:],
                    idxs_ap=idx2w[:, 36 * cg:36 * (cg + 1), :],
                    num_idxs=4608, num_idxs_reg=4608, elem_size=256,
                    single_packet=False)
                u4 = u4p.tile([128, 36, 64, 4], dt.float16)
                nc.vector.tensor_tensor(
                    u4[:],
                    g2[:].rearrange("p a (c k) -> p a c k", k=4),
                    w4h[:, 36 * cg:36 * (cg + 1), None, :].to_broadcast((128, 36, 64, 4)),
                    Alu.mult)
                ur = urp.tile([128, 2368], dt.float16)
                nc.vector.memset(ur[:, 2304:2368], 0.0)
                with nc.allow_low_precision("fp16 4-corner sum"):
                    nc.vector.tensor_reduce(
                        ur[:, 0:2304].rearrange("p (a c) -> p a c", c=64),
                        u4[:].rearrange("p a c k -> p (a c) k"),
                        axis=mybir.AxisListType.X, op=Alu.add)
                xt = xtp.tile([128, 5, 512], dt.float16)
                for bb in range(4):
                    for t in range(5):
                        nc.sync.dma_start(
                            xt[:, t, bb * 128:(bb + 1) * 128],
                            ur[:, bb * 576 + t * 128: bb * 576 + (t + 1) * 128],
                            transpose=True)
                ps = psm.tile([64, 512], f32)
                for t in range(5):
                    nc.tensor.matmul(ps[:], lhsT=w2[:, t * 64:(t + 1) * 64],
                                     rhs=xt[:, t, :], start=(t == 0), stop=(t == 4))
                osb = osp.tile([64, 512], f32)
                nc.scalar.copy(osb[:], ps[:])
                nc.sync.dma_start(out_d[:, cg * 512:(cg + 1) * 512], osb[:])

    nc.compile()
    return nc


def _get_program():
    if "nc" not in _CACHE:
        _CACHE["nc"] = _build_program()
    return _CACHE["nc"]


# ---------------------------------------------------------------------------
# host prep
# ---------------------------------------------------------------------------
def _prep_image(x_img, depth_img):
    """x_img (64,128,128) f32, depth_img (128,128) f32 -> (r2, r1)."""
    x_pad = np.pad(x_img, ((0, 0), (1, 1), (1, 1)))
    xp2 = np.pad(x_pad, ((0, 0), (0, 1), (0, 1)))          # (64,131,131)
    xhwc = np.ascontiguousarray(np.transpose(xp2, (1, 2, 0)))  # (131,131,64)
    r2 = np.empty((WP, WP, 64, 4), np.float16)
    r2[..., 0] = xhwc[:WP, :WP]
    r2[..., 1] = xhwc[:WP, 1:WP + 1]
    r2[..., 2] = xhwc[1:WP + 1, :WP]
    r2[..., 3] = xhwc[1:WP + 1, 1:WP + 1]
    r2 = r2.reshape(NREC, 256)

    d_pad = np.pad(depth_img, ((1, 1), (1, 1)))
    dp2 = np.pad(d_pad, ((0, 1), (0, 1)))                  # (131,131)
    r1 = np.zeros((WP, WP, 64), np.float32)
    r1[..., 0] = dp2[:WP, :WP]
    r1[..., 1] = dp2[:WP, 1:WP + 1]
    r1[..., 2] = dp2[1:WP + 1, :WP]
    r1[..., 3] = dp2[1:WP + 1, 1:WP + 1]
    return r2, r1.reshape(NREC, 64), x_pad


def kernel(x, depth, w_p, b_p, w_conv):
    from concourse.bass_utils import run_bass_kernel_spmd

    x = np.asarray(x, np.float32)
    depth = np.asarray(depth, np.float32)
    w_p = np.asarray(w_p, np.float32)
    b_p = np.asarray(b_p, np.float32)
    w_conv = np.asarray(w_conv, np.float32)

    nc = _get_program()

    # weights, shared
    wp_t = np.zeros((65, 9, 18), np.float32)
    for k in range(9):
        wp_t[:64, k, :] = w_p[:, :, k // 3, k % 3].T
    wp_t[64, 4, :] = b_p
    wp_t = wp_t.reshape(65, 162)

    W2 = np.transpose(w_conv.reshape(64, 64, 9), (2, 1, 0)).reshape(576, 64)
    W2p = np.zeros((640, 64), np.float32)
    W2p[:576] = W2
    w2_t = np.ascontiguousarray(
        W2p.reshape(5, 128, 64).transpose(1, 0, 2).reshape(128, 320)).astype(np.float16)

    pn_x = np.repeat(np.arange(-1, 2), 3).astype(np.float32)
    pn_y = np.tile(np.arange(-1, 2), 3).astype(np.float32)

    in_maps = []
    per_img = {}
    for img in range(B):
        per_img[img] = _prep_image(x[img], depth[img, 0])
    for core in range(8):
        img, st = divmod(core, 4)
        r0 = st * SP
        r2, r1, x_pad = per_img[img]
        xs = np.empty((65, 34, WP), np.float32)
        xs[:64] = x_pad[:, r0:r0 + 34, :]
        xs[64] = 1.0
        base = np.empty((128, 32, 18), np.float32)
        rows = (r0 + np.arange(32, dtype=np.float32) + 1.0)
        cols = (np.arange(128, dtype=np.float32) + 1.0)
        base[:, :, 0:9] = rows[None, :, None] + pn_x[None, None, :]
        base[:, :, 9:18] = cols[:, None, None] + pn_y[None, None, :]
        dcen = np.ascontiguousarray(depth[img, 0, r0:r0 + 32, :].T)
        in_maps.append({
            "xs": xs.reshape(65, 34 * WP),
            "r2": r2,
            "r1": r1,
            "base": base.reshape(128, 32 * 18),
            "dcen": dcen,
            "wp": wp_t,
            "w2": w2_t,
        })

    res = run_bass_kernel_spmd(nc, in_maps, core_ids=list(range(8)))
    out = np.empty((B, 64, H, W), np.float32)
    for core in range(8):
        img, st = divmod(core, 4)
        out[img, :, st * SP:(st + 1) * SP, :] = \
            res.results[core]["o"].reshape(64, SP, W)
    return out


# revision 11
# speedup vs baseline: 372.0693x; 372.0693x over previous
"""Deformable-conv (depth-aware) Trainium2 kernel.

Sharding: pure data parallel — 8 cores = 2 images x 4 H-strips of 32 rows.
Each core computes its strip's output from per-image gather-record tables.

Device algorithm per core (strip of 32 rows x 128 cols = 4096 pixels, 9
samples each):
  1. offset conv (PE): off[pix, 18] = sum_k x_slice @ w_p_k   (K=65 incl bias)
  2. pass-1 depth bilinear sampling via dma_gather of 2x2-block records
     (f32), with clamp-corrected row/col weights; depth weights dw, m (ACT exp)
  3. off2 = off * dw; pass-2 coords/weights; final per-corner weights w4 = m*row*col
  4. dma_gather of 2x2x64ch x-records (fp16, channel-major/corner-minor),
     one DVE mul (weights broadcast over channels) + corner-reduce
  5. DMA-transpose to [(n,c), pix] tiles, PE matmul vs w_conv -> out strip
"""
import numpy as np

B, C, H, W = 2, 64, 128, 128
N = 9
WP = W + 2           # 130 padded width
SP = H // 4          # 32 strip rows
NPIX = SP * W        # 4096 pixels per strip
NS = NPIX * N        # 36864 samples per strip
NREC = WP * WP       # 16900 records

_CACHE = {}


# ---------------------------------------------------------------------------
# device program
# ---------------------------------------------------------------------------
def _build_program():
    import concourse.bacc as bacc
    import concourse.tile as tile
    import concourse.mybir as mybir

    dt = mybir.dt
    Alu = mybir.AluOpType
    Act = mybir.ActivationFunctionType

    nc = bacc.Bacc("TRN2", target_bir_lowering=False, debug=False,
                   enable_asserts=False, num_devices=8)

    xs_d = nc.dram_tensor("xs", [65, 34 * WP], dt.float32, kind="ExternalInput")
    r2_d = nc.dram_tensor("r2", [NREC, 256], dt.float16, kind="ExternalInput")
    r1_d = nc.dram_tensor("r1", [NREC, 64], dt.float32, kind="ExternalInput")
    base_d = nc.dram_tensor("base", [128, 32 * 18], dt.float32, kind="ExternalInput")
    dcen_d = nc.dram_tensor("dcen", [128, 32], dt.float32, kind="ExternalInput")
    wp_d = nc.dram_tensor("wp", [65, 9 * 18], dt.float32, kind="ExternalInput")
    w2_d = nc.dram_tensor("w2", [128, 5 * 64], dt.float16, kind="ExternalInput")
    out_d = nc.dram_tensor("o", [64, NPIX], dt.float32, kind="ExternalOutput")

    HR = 16          # rows per pipeline stage (half strip)
    NHALF = SP // HR
    NRW = HR * 9     # idx rows per half (144)

    with tile.TileContext(nc) as tc:
        with (
            tc.tile_pool(name="const", bufs=1) as cp,
            tc.tile_pool(name="work", bufs=2) as wk,
            tc.tile_pool(name="g1p", bufs=2) as g1p,
            tc.tile_pool(name="g2p", bufs=2) as g2p,
            tc.tile_pool(name="u4p", bufs=1) as u4p,
            tc.tile_pool(name="t1p", bufs=1) as t1p,
            tc.tile_pool(name="pstp", bufs=4, space="PSUM") as pstp,
            tc.tile_pool(name="urp", bufs=2) as urp,
            tc.tile_pool(name="xtp", bufs=2) as xtp,
            tc.tile_pool(name="osp", bufs=2) as osp,
            tc.tile_pool(name="psc", bufs=2, space="PSUM") as psc,
            tc.tile_pool(name="psm", bufs=2, space="PSUM") as psm,
        ):
            f32 = dt.float32
            # ---- constants
            xs = cp.tile([65, 34, WP], f32, tag="xs")
            nc.sync.dma_start(xs[:], xs_d[:].rearrange("c (a b) -> c a b", b=WP))
            base = cp.tile([128, 32, 18], f32, tag="base")
            nc.sync.dma_start(base[:], base_d[:].rearrange("p (a b) -> p a b", b=18))
            dcen = cp.tile([128, 32], f32, tag="dcen")
            nc.sync.dma_start(dcen[:], dcen_d[:])
            wp = cp.tile([65, 9 * 18], f32, tag="wp")
            nc.sync.dma_start(wp[:], wp_d[:])
            w2 = cp.tile([128, 5 * 64], dt.float16, tag="w2")
            nc.sync.dma_start(w2[:], w2_d[:])
            ident = cp.tile([128, 128], dt.float16, tag="ident")
            from concourse.masks import make_identity
            make_identity(nc, ident[:])

            def sample_math(Pc, bound):
                fi = wk.tile([128, HR, 18], dt.int32, tag="sm_fi")
                nc.vector.tensor_copy(fi[:], Pc[:])
                f = wk.tile([128, HR, 18], f32, tag="sm_f")
                nc.vector.tensor_copy(f[:], fi[:])
                gt = wk.tile([128, HR, 18], f32, tag="sm_gt")
                nc.vector.tensor_tensor(gt[:], f[:], Pc[:], Alu.is_gt)
                nc.vector.tensor_sub(f[:], f[:], gt[:])
                qlt = wk.tile([128, HR, 18], f32, tag="sm_qlt")
                nc.vector.tensor_scalar(qlt[:], f[:], 0.0, float(bound - 1), Alu.max, Alu.min)
                qrb = wk.tile([128, HR, 18], f32, tag="sm_qrb")
                nc.vector.tensor_scalar(qrb[:], f[:], 1.0, float(bound - 1), Alu.add, Alu.min)
                nc.vector.tensor_scalar(qrb[:], qrb[:], 0.0, None, Alu.max)
                pc = wk.tile([128, HR, 18], f32, tag="sm_pc")
                nc.vector.tensor_scalar(pc[:], Pc[:], 0.0, float(bound - 1), Alu.max, Alu.min)
                gl = wk.tile([128, HR, 18], f32, tag="sm_gl")
                nc.vector.scalar_tensor_tensor(gl[:], qlt[:], 1.0, pc[:], Alu.add, Alu.subtract)
                gr = wk.tile([128, HR, 18], f32, tag="sm_gr")
                nc.vector.scalar_tensor_tensor(gr[:], pc[:], 1.0, qrb[:], Alu.add, Alu.subtract)
                r0 = wk.tile([128, HR, 18], f32, tag="sm_r0")
                nc.vector.tensor_scalar(r0[:], qlt[:], 0.0, float(bound - 2), Alu.max, Alu.min)
                r0p = wk.tile([128, HR, 18], f32, tag="sm_r0p")
                nc.vector.tensor_scalar(r0p[:], r0[:], 1.0, None, Alu.add)
                eq = wk.tile([128, HR, 18], f32, tag="sm_eq")
                wA = wk.tile([128, HR, 18], f32, tag="sm_wA")
                wB = wk.tile([128, HR, 18], f32, tag="sm_wB")
                tmp = wk.tile([128, HR, 18], f32, tag="sm_tmp")
                nc.vector.tensor_tensor(eq[:], qlt[:], r0[:], Alu.is_equal)
                nc.vector.tensor_mul(wA[:], gl[:], eq[:])
                nc.vector.tensor_tensor(eq[:], qrb[:], r0[:], Alu.is_equal)
                nc.vector.tensor_mul(tmp[:], gr[:], eq[:])
                nc.vector.tensor_add(wA[:], wA[:], tmp[:])
                nc.vector.tensor_tensor(eq[:], qlt[:], r0p[:], Alu.is_equal)
                nc.vector.tensor_mul(wB[:], gl[:], eq[:])
                nc.vector.tensor_tensor(eq[:], qrb[:], r0p[:], Alu.is_equal)
                nc.vector.tensor_mul(tmp[:], gr[:], eq[:])
                nc.vector.tensor_add(wB[:], wB[:], tmp[:])
                return r0, wA, wB

            def make_idx(r0, name):
                idxf = wk.tile([128, HR, 9], f32, tag=name + "_f")
                nc.vector.scalar_tensor_tensor(
                    idxf[:], r0[:, :, 0:9], float(WP), r0[:, :, 9:18],
                    Alu.mult, Alu.add)
                idxi = wk.tile([128, NRW], dt.int16, tag=name + "_i")
                nc.vector.tensor_copy(idxi[:], idxf[:].rearrange("p a b -> p (a b)"))
                idxw = wk.tile([128, NRW, 8], dt.int16, tag=name + "_w")
                for s in range(8):
                    nc.sync.dma_start(idxw[0:16, :, s], idxi[16 * s:16 * (s + 1), :])
                nc.sync.dma_start(idxw[16:32, :, :], idxw[0:16, :, :])
                nc.sync.dma_start(idxw[32:64, :, :], idxw[0:32, :, :])
                nc.sync.dma_start(idxw[64:128, :, :], idxw[0:64, :, :])
                return idxw

            for hf in range(NHALF):
                rbase = hf * HR
                # ---- stage A: offset conv -> OFF [128, HR, 18]
                OFF = wk.tile([128, HR, 18], f32, tag="OFF")
                for bg in range(HR // 4):
                    ps = psc.tile([128, 72], f32)
                    for bb in range(4):
                        b = rbase + bg * 4 + bb
                        for k in range(9):
                            drr, dcc = k // 3, k % 3
                            nc.tensor.matmul(
                                ps[:, bb * 18:(bb + 1) * 18],
                                lhsT=xs[:, b + drr, dcc:dcc + 128],
                                rhs=wp[:, k * 18:(k + 1) * 18],
                                start=(k == 0), stop=(k == 8),
                            )
                    nc.scalar.copy(OFF[:, bg * 4:(bg + 1) * 4, :],
                                   ps[:].rearrange("p (a b) -> p a b", b=18))

                bsl = base[:, rbase:rbase + HR, :]
                # ---- stage B: pass-1 coords/weights
                P1 = wk.tile([128, HR, 18], f32, tag="P1")
                nc.vector.tensor_add(P1[:], OFF[:], bsl)
                r0_1, wA1, wB1 = sample_math(P1, H)
                wT1 = wk.tile([128, HR, 9], f32, tag="wT1")
                nc.vector.tensor_copy(wT1[:], wA1[:, :, 0:9])
                wB1x = wk.tile([128, HR, 9], f32, tag="wB1x")
                nc.vector.tensor_copy(wB1x[:], wB1[:, :, 0:9])
                cL1 = wk.tile([128, HR, 9], f32, tag="cL1")
                nc.vector.tensor_copy(cL1[:], wA1[:, :, 9:18])
                cR1 = wk.tile([128, HR, 9], f32, tag="cR1")
                nc.vector.tensor_copy(cR1[:], wB1[:, :, 9:18])
                idx1w = make_idx(r0_1, "idx1")

                # ---- stage D: pass-1 gathers + blend
                DOFF = wk.tile([128, HR, 9], f32, tag="DOFF")
                for ch in range(HR // 4):
                    g1 = g1p.tile([128, 36, 64], f32)
                    nc.gpsimd.dma_gather(
                        out_ap=g1[:], in_ap=r1_d[:],
                        idxs_ap=idx1w[:, 36 * ch:36 * (ch + 1), :],
                        num_idxs=4608, num_idxs_reg=4608, elem_size=64,
                        single_packet=False)
                    sl = slice(4 * ch, 4 * (ch + 1))
                    a = wk.tile([128, 4, 9], f32, tag="p1_a")
                    bt = wk.tile([128, 4, 9], f32, tag="p1_b")
                    t2 = wk.tile([128, 4, 9], f32, tag="p1_t")
                    ga = g1[:].rearrange("p (a b) c -> p a b c", b=9)
                    nc.vector.tensor_mul(a[:], ga[:, :, :, 0], cL1[:, sl, :])
                    nc.vector.tensor_mul(t2[:], ga[:, :, :, 1], cR1[:, sl, :])
                    nc.vector.tensor_add(a[:], a[:], t2[:])
                    nc.vector.tensor_mul(bt[:], ga[:, :, :, 2], cL1[:, sl, :])
                    nc.vector.tensor_mul(t2[:], ga[:, :, :, 3], cR1[:, sl, :])
                    nc.vector.tensor_add(bt[:], bt[:], t2[:])
                    nc.vector.tensor_mul(a[:], a[:], wT1[:, sl, :])
                    nc.vector.tensor_mul(bt[:], bt[:], wB1x[:, sl, :])
                    nc.vector.tensor_add(DOFF[:, sl, :], a[:], bt[:])

                # ---- stage E: depth weights
                dd = wk.tile([128, HR, 9], f32, tag="dd")
                nc.vector.tensor_sub(
                    dd[:], dcen[:, rbase:rbase + HR, None].to_broadcast((128, HR, 9)),
                    DOFF[:])
                nc.scalar.activation(dd[:], dd[:], Act.Abs)
                dw = wk.tile([128, HR, 9], f32, tag="dw")
                nc.scalar.activation(dw[:], dd[:], Act.Exp, scale=-4.0)
                nc.vector.tensor_scalar(dw[:], dw[:], 0.25, None, Alu.add)
                mm = wk.tile([128, HR, 9], f32, tag="mm")
                nc.scalar.activation(mm[:], dd[:], Act.Exp, scale=-1.0)

                # ---- stage F: pass-2 coords/weights/indices
                P2 = wk.tile([128, HR, 18], f32, tag="P2")
                nc.vector.tensor_mul(P2[:, :, 0:9], OFF[:, :, 0:9], dw[:])
                nc.vector.tensor_mul(P2[:, :, 9:18], OFF[:, :, 9:18], dw[:])
                nc.vector.tensor_add(P2[:], P2[:], bsl)
                r0_2, wA2, wB2 = sample_math(P2, H + 2)
                idx2w = make_idx(r0_2, "idx2")
                wTm = wk.tile([128, HR, 9], f32, tag="wTm")
                nc.vector.tensor_mul(wTm[:], wA2[:, :, 0:9], mm[:])
                wBm = wk.tile([128, HR, 9], f32, tag="wBm")
                nc.vector.tensor_mul(wBm[:], wB2[:, :, 0:9], mm[:])
                w4 = wk.tile([128, NRW, 4], f32, tag="w4")
                w4v = w4[:].rearrange("p (a b) c -> p a b c", b=9)
                nc.vector.tensor_mul(w4v[:, :, :, 0], wTm[:], wA2[:, :, 9:18])
                nc.vector.tensor_mul(w4v[:, :, :, 1], wTm[:], wB2[:, :, 9:18])
                nc.vector.tensor_mul(w4v[:, :, :, 2], wBm[:], wA2[:, :, 9:18])
                nc.vector.tensor_mul(w4v[:, :, :, 3], wBm[:], wB2[:, :, 9:18])
                w4h2 = wk.tile([128, NRW, 4, 2], dt.float16, tag="w4h2")
                nc.vector.tensor_copy(
                    w4h2[:], w4[:, :, :, None].to_broadcast((128, NRW, 4, 2)))

                # ---- stage G: pass-2 gather, blend, transpose, matmul
                for cg in range(HR // 4):
                    g2 = g2p.tile([128, 36, 256], dt.float16)
                    nc.gpsimd.dma_gather(
                        out_ap=g2[:], in_ap=r2_d[:],
                        idxs_ap=idx2w[:, 36 * cg:36 * (cg + 1), :],
                        num_idxs=4608, num_idxs_reg=4608, elem_size=256,
                        single_packet=False)
                    u4 = u4p.tile([128, 36, 32, 4, 2], dt.float16)
                    nc.vector.tensor_tensor(
                        u4[:],
                        g2[:].rearrange("p a (h k l) -> p a h k l", k=4, l=2),
                        w4h2[:, 36 * cg:36 * (cg + 1), None, :, :].to_broadcast(
                            (128, 36, 32, 4, 2)),
                        Alu.mult)
                    u4v = u4[:].rearrange("p a h k l -> p (a h) k l")
                    t1 = t1p.tile([128, 1152, 2, 2], dt.float16)
                    nc.vector.tensor_tensor(t1[:], u4v[:, :, 0:2, :], u4v[:, :, 2:4, :], Alu.add)
                    ur = urp.tile([128, 2368], dt.float16)
                    nc.vector.memset(ur[:, 2304:2368], 0.0)
                    urv = ur[:, 0:2304].rearrange("p (a l) -> p a l", l=2)
                    nc.vector.tensor_tensor(urv, t1[:, :, 0, :], t1[:, :, 1, :], Alu.add)
                    xt = xtp.tile([128, 5, 512], dt.float16)
                    for bb in range(4):
                        for t in range(5):
                            pst = pstp.tile([128, 128], dt.float16, space="PSUM")
                            nc.tensor.transpose(
                                pst[:],
                                ur[:, bb * 576 + t * 128: bb * 576 + (t + 1) * 128],
                                ident[:])
                            nc.scalar.copy(xt[:, t, bb * 128:(bb + 1) * 128], pst[:])
                    ps = psm.tile([64, 512], f32)
                    for t in range(5):
                        nc.tensor.matmul(ps[:], lhsT=w2[:, t * 64:(t + 1) * 64],
                                         rhs=xt[:, t, :], start=(t == 0), stop=(t == 4))
                    osb = osp.tile([64, 512], f32)
                    nc.scalar.copy(osb[:], ps[:])
                    off0 = (rbase + cg * 4) * 128
                    nc.sync.dma_start(out_d[:, off0:off0 + 512], osb[:])

    nc.compile()
    return nc


def _get_program():
    if "nc" not in _CACHE:
        _CACHE["nc"] = _build_program()
    return _CACHE["nc"]


# ---------------------------------------------------------------------------
# host prep
# ---------------------------------------------------------------------------
def _prep_image(x_img, depth_img):
    """x_img (64,128,128) f32, depth_img (128,128) f32 -> (r2, r1)."""
    x_pad = np.pad(x_img, ((0, 0), (1, 1), (1, 1)))
    xp2 = np.pad(x_pad, ((0, 0), (0, 1), (0, 1)))          # (64,131,131)
    xhwc = np.ascontiguousarray(np.transpose(xp2, (1, 2, 0)))  # (131,131,64)
    r2 = np.empty((WP, WP, 64, 4), np.float16)
    r2[..., 0] = xhwc[:WP, :WP]
    r2[..., 1] = xhwc[:WP, 1:WP + 1]
    r2[..., 2] = xhwc[1:WP + 1, :WP]
    r2[..., 3] = xhwc[1:WP + 1, 1:WP + 1]
    # record layout [c//2, corner, c%2] so both the weight-mul and the
    # corner-pair adds hit the DVE 2x packed mode
    r2 = np.ascontiguousarray(
        r2.reshape(WP, WP, 32, 2, 4).transpose(0, 1, 2, 4, 3)).reshape(NREC, 256)

    d_pad = np.pad(depth_img, ((1, 1), (1, 1)))
    dp2 = np.pad(d_pad, ((0, 1), (0, 1)))                  # (131,131)
    r1 = np.zeros((WP, WP, 64), np.float32)
    r1[..., 0] = dp2[:WP, :WP]
    r1[..., 1] = dp2[:WP, 1:WP + 1]
    r1[..., 2] = dp2[1:WP + 1, :WP]
    r1[..., 3] = dp2[1:WP + 1, 1:WP + 1]
    return r2, r1.reshape(NREC, 64), x_pad


def kernel(x, depth, w_p, b_p, w_conv):
    from concourse.bass_utils import run_bass_kernel_spmd

    x = np.asarray(x, np.float32)
    depth = np.asarray(depth, np.float32)
    w_p = np.asarray(w_p, np.float32)
    b_p = np.asarray(b_p, np.float32)
    w_conv = np.asarray(w_conv, np.float32)

    nc = _get_program()

    # weights, shared
    wp_t = np.zeros((65, 9, 18), np.float32)
    for k in range(9):
        wp_t[:64, k, :] = w_p[:, :, k // 3, k % 3].T
    wp_t[64, 4, :] = b_p
    wp_t = wp_t.reshape(65, 162)

    W2 = np.transpose(w_conv.reshape(64, 64, 9), (2, 1, 0)).reshape(576, 64)
    W2p = np.zeros((640, 64), np.float32)
    W2p[:576] = W2
    w2_t = np.ascontiguousarray(
        W2p.reshape(5, 128, 64).transpose(1, 0, 2).reshape(128, 320)).astype(np.float16)

    pn_x = np.repeat(np.arange(-1, 2), 3).astype(np.float32)
    pn_y = np.tile(np.arange(-1, 2), 3).astype(np.float32)

    in_maps = []
    per_img = {}
    for img in range(B):
        per_img[img] = _prep_image(x[img], depth[img, 0])
    for core in range(8):
        img, st = divmod(core, 4)
        r0 = st * SP
        r2, r1, x_pad = per_img[img]
        xs = np.empty((65, 34, WP), np.float32)
        xs[:64] = x_pad[:, r0:r0 + 34, :]
        xs[64] = 1.0
        base = np.empty((128, 32, 18), np.float32)
        rows = (r0 + np.arange(32, dtype=np.float32) + 1.0)
        cols = (np.arange(128, dtype=np.float32) + 1.0)
        base[:, :, 0:9] = rows[None, :, None] + pn_x[None, None, :]
        base[:, :, 9:18] = cols[:, None, None] + pn_y[None, None, :]
        dcen = np.ascontiguousarray(depth[img, 0, r0:r0 + 32, :].T)
        in_maps.append({
            "xs": xs.reshape(65, 34 * WP),
            "r2": r2,
            "r1": r1,
            "base": base.reshape(128, 32 * 18),
            "dcen": dcen,
            "wp": wp_t,
            "w2": w2_t,
        })

    res = run_bass_kernel_spmd(nc, in_maps, core_ids=list(range(8)))
    out = np.empty((B, 64, H, W), np.float32)
    for core in range(8):
        img, st = divmod(core, 4)
        out[img, :, st * SP:(st + 1) * SP, :] = \
            res.results[core]["o"].reshape(64, SP, W)
    return out
